# revision 8
# baseline (speedup 1.0000x reference)
"""NT-Xent (SimCLR) contrastive loss on 8 Trainium2 NeuronCores, v2.

Symmetric-half version. The exp(similarity) matrix E is symmetric, so each
unordered pair {g, h} is computed ONCE and contributes to both rows' softmax
denominators: once via a row-sum (ACT accumulator) and once via a column-sum
(selector-matmul against the exp'd block). This halves both the PE matmul
volume and - more importantly - the ACT exp volume, which is the hard floor
of the dense version.

Decomposition (uniform across cores via host-side column rotation by
1024*c): core c owns global rows [1024c, 1024c+1024) = local columns
[0, 1024). For local row-tile rt (128 rows), it computes the band of
columns [rt*128, rt*128+4096) in four 1024-wide chunks, plus a 128-wide
"end piece" at [rt*128+4096, +128). A lower-triangle mask (-1e5, incl.
diagonal) on the first 128 columns and an upper-triangle mask on the end
piece make the covered set exactly {(i, j): 0 < (j-i) mod 8192 < 4096},
each unordered pair once. Pairs at distance exactly 4096 are the positive
pairs; their dot products are returned per-row and folded in on the host.

Per core outputs: rsums [128, 40] (row partial sums per (rt, chunk) +
end-piece), csums [5, 8, 1024] (column partial sums, selector-matmul
landing row-tile rt's strip on PSUM partition rt), posv [1, 1024]
(positive-pair dots). The host adds row + column partials into the global
denominator vector, takes ln, and forms the scalar loss - that is the
all-reduce step of the sharding hint, done once over 8k floats.

All matmul operands are bf16; normalization (sum of squares -> rsqrt via
ln/exp -> column scaling) runs on-device in bf16 exactly as in v1.
"""

import sys

for _p in ("/opt/trn_rl_repo",):
    if _p not in sys.path:
        sys.path.insert(0, _p)

import ml_dtypes
import numpy as np

import concourse.bass as bass
import concourse.tile as tile
from concourse import bacc, mybir
from concourse.bass_utils import run_bass_kernel_spmd

F32 = mybir.dt.float32
BF16 = mybir.dt.bfloat16
AF = mybir.ActivationFunctionType

N_CORES = 8
N = 4096
D = 256
TWO_N = 2 * N
ROWS = TWO_N // N_CORES   # 1024 rows per core
BAND = 4096               # band width per row-tile (exclusive of end piece)
NCH = BAND // 1024        # 4 chunks of 1024
COLS = ROWS + BAND        # 5120 local columns used per core
NEG_MASK = -1.0e5

_CACHE = {}
LAST_RESULTS = None


def _build_nc() -> bass.Bass:
    nc = bacc.Bacc("TRN2", num_devices=N_CORES)

    zt_d = nc.dram_tensor("zt", [D, COLS], BF16, kind="ExternalInput")
    dmask_d = nc.dram_tensor("dmaskT", [128, 128], BF16, kind="ExternalInput")
    emask_d = nc.dram_tensor("emaskT", [128, 128], BF16, kind="ExternalInput")
    ident_d = nc.dram_tensor("ident", [128, 128], BF16, kind="ExternalInput")
    sel_d = nc.dram_tensor("sel8", [128, 64], BF16, kind="ExternalInput")
    rsums_d = nc.dram_tensor("rsums", [128, 40], F32, kind="ExternalOutput")
    csums_d = nc.dram_tensor("csums", [5, 8 * 1024], F32, kind="ExternalOutput")
    posv_d = nc.dram_tensor("posv", [1, ROWS], F32, kind="ExternalOutput")
    u_d = nc.dram_tensor("uscratch", [1, COLS], BF16)  # DRAM bounce for u

    with tile.TileContext(nc) as tc:
        with (
            tc.tile_pool(name="big", bufs=1) as big,
            tc.tile_pool(name="wsq", bufs=4) as wsq,
            tc.tile_pool(name="wub", bufs=3) as wub,
            tc.tile_pool(name="wsm", bufs=6) as wsm,
            tc.tile_pool(name="wsr", bufs=2) as wsr,
            tc.tile_pool(name="wex", bufs=3) as wex,
            tc.tile_pool(name="small", bufs=1) as small,
            tc.tile_pool(name="stat", bufs=1) as stat,
            tc.tile_pool(name="ps", bufs=2, space="PSUM") as ps,
            tc.tile_pool(name="cs", bufs=2, space="PSUM") as cs,
        ):
            zt0 = big.tile([128, COLS], BF16, tag="zt0")   # dims 0:128, scaled
            zt1 = big.tile([128, COLS], BF16, tag="zt1")   # dims 128:256, scaled
            ztr0 = big.tile([128, COLS], BF16, tag="ztr0")  # raw
            ztr1 = big.tile([128, COLS], BF16, tag="ztr1")
            dmaskT = small.tile([128, 128], BF16, tag="dmaskT")
            nc.sync.dma_start(out=dmaskT[:, :], in_=dmask_d.ap()[:, :])
            emaskT = small.tile([128, 128], BF16, tag="emaskT")
            nc.sync.dma_start(out=emaskT[:, :], in_=emask_d.ap()[:, :])
            ident = small.tile([128, 128], BF16, tag="ident")
            nc.sync.dma_start(out=ident[:, :], in_=ident_d.ap()[:, :])
            sel = small.tile([128, 64], BF16, tag="sel")
            nc.sync.dma_start(out=sel[:, :], in_=sel_d.ap()[:, :])
            ones_f = small.tile([128, 1], F32, tag="onesf")
            nc.vector.memset(ones_f[:, :], 1.0)
            ones = small.tile([128, 1], BF16, tag="ones")
            nc.vector.tensor_copy(ones[:, :], ones_f[:, :])

            # rsums[:, rt*4+k] = chunk (rt, k); rsums[:, 32+rt] = end piece
            rsums = stat.tile([128, 40], F32, tag="rsum")

            # norm blocks: one big 2048 block first (colpass(0) only needs
            # cols < 1920, so a single prologue chain gates it), then 1024s
            NBLOCKS = [2048, 1024, 1024, 1024]
            NOFFS = [0, 2048, 3072, 4096]

            def load(b):
                sl = slice(NOFFS[b], NOFFS[b] + NBLOCKS[b])
                nc.sync.dma_start(out=ztr0[:, sl], in_=zt_d.ap()[0:128, sl])
                nc.sync.dma_start(out=ztr1[:, sl], in_=zt_d.ap()[128:256, sl])

            def prologue(b):
                o, w = NOFFS[b], NBLOCKS[b]
                sl = slice(o, o + w)
                sq0 = wsq.tile([128, w], BF16, tag="sq")
                sqs = wsq.tile([128, w], BF16, tag="sq")
                nc.vector.tensor_mul(sq0[:, :], ztr0[:, sl], ztr0[:, sl])
                # pre-added squares keep the norm matmul count at one per
                # 512 columns
                nc.vector.tensor_mul(sqs[:, :], ztr1[:, sl], ztr1[:, sl])
                nc.vector.tensor_add(sqs[:, :], sqs[:, :], sq0[:, :])
                # reduce + rsqrt in pipelined 1024-wide sub-chunks (a psum
                # strip wider than 1024 won't fit beside the pools)
                for so in range(0, w, 1024):
                    sw = min(1024, w - so)
                    nrm = cs.tile([8, 1024], F32, tag="cs")
                    for bb in range(sw // 512):
                        bs = slice(so + bb * 512, so + (bb + 1) * 512)
                        nc.tensor.matmul(
                            nrm[0:1, bb * 512 : (bb + 1) * 512],
                            ones[:, :], sqs[:, bs],
                            start=True, stop=True,
                        )
                    ssqr = wsr.tile([1, sw], F32, tag="ssqr")
                    nc.vector.tensor_copy(ssqr[0:1, :], nrm[0:1, 0:sw])
                    np_ = sw // 64
                    sstb = wsm.tile([np_, 64], F32, tag="sstb")
                    nc.sync.dma_start(out=sstb[0:np_, :], in_=ssqr[0:1, :])
                    # u = exp(-0.5*ln(ssq)); single Ln+Exp ACT table set
                    lnt = wsm.tile([np_, 64], F32, tag="lnt")
                    ut = wsm.tile([np_, 64], BF16, tag="ut")
                    nc.scalar.activation(lnt[:, :], sstb[0:np_, :], AF.Ln)
                    nc.scalar.activation(ut[:, :], lnt[:, :], AF.Exp, scale=-0.5)
                    u_out = bass.AP(
                        tensor=u_d.ap().tensor, offset=o + so,
                        ap=[[64, np_], [1, 64]],
                    )
                    nc.sync.dma_start(out=u_out, in_=ut[:, :])
                ubc = wub.tile([128, w], BF16, tag="ubc")
                u_sl = u_d.ap()[0:1, sl]
                u_bcast = bass.AP(
                    tensor=u_sl.tensor,
                    offset=u_sl.offset,
                    ap=[[0, 128]] + list(u_sl.ap[1:]),
                )
                nc.sync.dma_start(out=ubc[:, :], in_=u_bcast)
                return ubc

            def mults(b, ubc):
                sl = slice(NOFFS[b], NOFFS[b] + NBLOCKS[b])
                nc.vector.tensor_mul(zt0[:, sl], ztr0[:, sl], ubc[:, :])
                nc.vector.tensor_mul(zt1[:, sl], ztr1[:, sl], ubc[:, :])

            def colpass(k):
                # chunk k of every row-tile's band: cols [rt*128+k*1024, +1024)
                cs8 = cs.tile([8, 1024], F32, tag="cs")
                pending = []  # exp'd tiles whose cs matmuls are deferred

                def flush_cs():
                    prt, pex = pending.pop(0)
                    # column sums of E land on psum partition prt via the
                    # selector matmul; accumulated over rt in one psum tile
                    for bb in range(2):
                        bs = slice(bb * 512, (bb + 1) * 512)
                        nc.tensor.matmul(
                            cs8[:, bs],
                            sel[:, prt * 8 : prt * 8 + 8],
                            pex[:, bs],
                            start=(prt == 0),
                            stop=(prt == 7),
                        )

                for rt in range(8):
                    r0 = rt * 128
                    o = r0 + k * 1024
                    pq = ps.tile([128, 1024], F32, tag="mm")
                    for ki, zk in enumerate((zt0, zt1)):
                        lhsT = zk[:, r0 : r0 + 128]
                        for bb in range(2):
                            last = ki == 1 and not (k == 0 and bb == 0)
                            nc.tensor.matmul(
                                pq[:, bb * 512 : (bb + 1) * 512],
                                lhsT,
                                zk[:, o + bb * 512 : o + (bb + 1) * 512],
                                start=(ki == 0),
                                stop=last,
                            )
                    if k == 0:
                        # mask self + strictly-lower triangle (those pairs
                        # belong to earlier rows' bands) by accumulating
                        # maskT.T @ I on the PE - keeps DVE off this path
                        nc.tensor.matmul(
                            pq[:, 0:128], dmaskT[:, :], ident[:, :],
                            start=False, stop=True,
                        )
                    # cs matmuls for the PREVIOUS row-tile go after this
                    # tile's main matmuls: they wait on that tile's exp
                    # (ACT), and issuing them one tile late keeps the PE
                    # queue non-empty while ACT catches up.
                    if pending:
                        flush_cs()
                    ex = wex.tile([128, 1024], BF16, tag="ex")
                    nc.scalar.activation(
                        ex[:, :], pq[:, :], AF.Exp, scale=2.0,
                        accum_out=rsums[:, rt * 4 + k : rt * 4 + k + 1],
                    )
                    pending.append((rt, ex))
                while pending:
                    flush_cs()
                css = wsr.tile([8, 1024], F32, tag="css")
                nc.vector.tensor_copy(css[:, :], cs8[:, :])
                nc.sync.dma_start(
                    out=csums_d.ap()[k : k + 1, :], in_=css[:, :]
                )

            def endpass():
                # 128-wide end pieces at cols [rt*128+4096, +128), all 8
                # row-tiles batched into one [128, 1024] psum tile.
                cs8 = cs.tile([8, 1024], F32, tag="cs")
                pe = ps.tile([128, 1024], F32, tag="mm")
                for rt in range(8):
                    r0 = rt * 128
                    o = r0 + BAND
                    for ki, zk in enumerate((zt0, zt1)):
                        nc.tensor.matmul(
                            pe[:, r0 : r0 + 128],
                            zk[:, r0 : r0 + 128],
                            zk[:, o : o + 128],
                            start=(ki == 0),
                            stop=False,
                        )
                    # keep only strictly-lower (j-i in (4096-p, 4095]);
                    # the rest gets -1e5 via the PE-accumulated mask
                    nc.tensor.matmul(
                        pe[:, r0 : r0 + 128], emaskT[:, :], ident[:, :],
                        start=False, stop=True,
                    )
                ex = wex.tile([128, 1024], BF16, tag="ex")
                nc.scalar.activation(ex[:, :], pe[:, :], AF.Exp, scale=2.0)
                # per-piece row sums: reduce each 128-col group
                nc.vector.tensor_reduce(
                    rsums[:, 32:40],
                    ex[:, :].rearrange("p (r q) -> p r q", q=128),
                    axis=mybir.AxisListType.X,
                    op=mybir.AluOpType.add,
                )
                for rt in range(8):
                    bs = slice(rt * 128, rt * 128 + 128)
                    # disjoint psum regions: each is its own accum group
                    nc.tensor.matmul(
                        cs8[:, bs],
                        sel[:, rt * 8 : rt * 8 + 8],
                        ex[:, bs],
                        start=True,
                        stop=True,
                    )
                css = wsr.tile([8, 1024], F32, tag="css")
                nc.vector.tensor_copy(css[:, :], cs8[:, :])
                nc.sync.dma_start(out=csums_d.ap()[4:5, :], in_=css[:, :])

            def pos_pass():
                # pos_dot[i] = zn_i . zn_{i+4096}, local rows 0..1023
                posps = cs.tile([8, 1024], F32, tag="cs")
                for ki, zk in enumerate((zt0, zt1)):
                    prod = wsq.tile([128, ROWS], BF16, tag="sq")
                    nc.vector.tensor_mul(
                        prod[:, :], zk[:, 0:ROWS], zk[:, BAND : BAND + ROWS]
                    )
                    for bb in range(2):
                        bs = slice(bb * 512, (bb + 1) * 512)
                        nc.tensor.matmul(
                            posps[0:1, bs], ones[:, :], prod[:, bs],
                            start=(ki == 0), stop=(ki == 1),
                        )
                pv = wsr.tile([1, ROWS], F32, tag="pv")
                nc.vector.tensor_copy(pv[0:1, :], posps[0:1, :])
                nc.sync.dma_start(out=posv_d.ap()[:, :], in_=pv[0:1, :])

            # Two-block normalization lookahead: each prologue's ACT
            # (ln/exp) and DMA steps are emitted a full column pass early,
            # so they never queue behind a colpass's eight big exps
            # (engines execute in emission order). Colpasses emit no DVE
            # ops (masks ride the PE), so the chains cannot interleave
            # badly. pos/end passes run before the last colpass so their
            # serial chains overlap its PE work instead of extending the
            # tail.
            load(0)
            load(1)
            load(2)
            ub = {}
            ub[0] = prologue(0)
            mults(0, ub[0])
            ub[1] = prologue(1)
            mults(1, ub[1])
            colpass(0)   # needs cols < 1920
            load(3)
            ub[2] = prologue(2)
            mults(2, ub[2])
            colpass(1)   # needs cols < 2944
            ub[3] = prologue(3)
            mults(3, ub[3])
            colpass(2)   # needs cols < 3968
            pos_pass()   # needs cols [0,1024) + [4096,5120)
            endpass()    # needs cols < 5120
            colpass(3)   # needs cols < 4992
            nc.sync.dma_start(out=rsums_d.ap()[:, :], in_=rsums[:, :])

    _combined_set_id = _act_set_id_with_exp_and_ln(nc)

    def _single_act_table_load():
        for blk in nc.main_func.blocks:
            insts = list(blk.instructions)
            for i, ins in enumerate(insts):
                if isinstance(ins, mybir.InstActivation):
                    load_i = mybir.InstLoadActFuncSet(
                        name=nc.get_next_instruction_name(),
                        act_func_set_id=_combined_set_id,
                        ins=[],
                        outs=[],
                    )
                    load_i.engine = mybir.EngineType.Activation
                    insts.insert(i, load_i)
                    blk.instructions = insts
                    break

    nc.insert_act_table_loads = _single_act_table_load
    nc.compile()
    return nc


def _act_set_id_with_exp_and_ln(nc) -> int:
    from concourse.hw_specs import get_activation_tables

    tabs = get_activation_tables(nc.m.arch)
    for i, (name, fns) in enumerate(tabs.items()):
        if AF.Exp in fns and AF.Ln in fns:
            return i
    raise RuntimeError("no activation table set with both Exp and Ln")


def _get_nc() -> bass.Bass:
    if "nc" not in _CACHE:
        _CACHE["nc"] = _build_nc()
    return _CACHE["nc"]


def _masks():
    # additive masks are applied on the PE as maskT.T @ I, so ship them
    # pre-transposed.
    q = np.arange(128)
    # dmask: kill self + strictly-lower triangle (q <= p)
    dm = np.where(q[None, :] <= q[:, None], NEG_MASK, 0.0)
    # emask piece: keep strictly-lower (q < p), kill the rest
    em = np.where(q[None, :] < q[:, None], 0.0, NEG_MASK)
    ident = np.eye(128, dtype=np.float32)
    # sel8: slice rt is a [128, 8] selector with column rt all-ones
    sel = np.zeros((128, 64), dtype=np.float32)
    for rt in range(8):
        sel[:, rt * 8 + rt] = 1.0
    bf = ml_dtypes.bfloat16
    return (
        np.ascontiguousarray(dm.T).astype(bf),
        np.ascontiguousarray(em.T).astype(bf),
        ident.astype(bf),
        sel.astype(bf),
    )


def kernel(emb_i: np.ndarray, emb_j: np.ndarray) -> np.ndarray:
    global LAST_RESULTS
    z = np.concatenate(
        [np.asarray(emb_i, dtype=np.float32), np.asarray(emb_j, dtype=np.float32)],
        axis=0,
    )  # [8192, 256]
    zt = np.ascontiguousarray(z.T).astype(ml_dtypes.bfloat16)  # [256, 8192]
    dmT, emT, ident, sel = _masks()

    in_maps = []
    for c in range(N_CORES):
        ztc = zt if c == 0 else np.roll(zt, -c * ROWS, axis=1)
        in_maps.append(
            {
                "zt": np.ascontiguousarray(ztc[:, :COLS]),
                "dmaskT": dmT,
                "emaskT": emT,
                "ident": ident,
                "sel8": sel,
            }
        )

    nc = _get_nc()
    LAST_RESULTS = run_bass_kernel_spmd(nc, in_maps, list(range(N_CORES)))

    denom = np.zeros(TWO_N, dtype=np.float64)
    pos_sum = 0.0
    for c, r in enumerate(LAST_RESULTS.results):
        rs = np.asarray(r["rsums"], dtype=np.float64)    # [128, 40]
        csv = np.asarray(r["csums"], dtype=np.float64)   # [5, 8192]
        pv = np.asarray(r["posv"], dtype=np.float64)[0]  # [1024]
        base = c * ROWS
        # row partials: row rt*128+p gets rsums[p, rt*4:rt*4+4] + rsums[p,32+rt]
        main = rs[:, :32].reshape(128, 8, 4).sum(axis=2)  # [p, rt]
        endp = rs[:, 32:40]                               # [p, rt]
        rows = (main + endp).T.reshape(-1)                # rt-major -> [1024]
        denom[base : base + ROWS] += rows
        # column partials: chunk strip (k, rt) covers local cols
        # [rt*128 + k*1024, +1024); end strip row 4, piece rt covers
        # [rt*128 + 4096, +128)
        cs5 = csv.reshape(5, 8, 1024)
        for k in range(4):
            for rt in range(8):
                lo = rt * 128 + k * 1024
                idx = (base + lo + np.arange(1024)) % TWO_N
                np.add.at(denom, idx, cs5[k, rt])
        for rt in range(8):
            lo = rt * 128 + BAND
            idx = (base + lo + np.arange(128)) % TWO_N
            np.add.at(denom, idx, cs5[4, rt, rt * 128 : rt * 128 + 128])
        # positive pairs: E[g, g+4096] = exp(2*d_g) joins the denominator
        denom[base : base + ROWS] += np.exp(2.0 * pv)
        pos_sum += pv.sum()

    # every global row's pos term is computed exactly once across cores;
    # pos_g = sim[g, g+N]/T = 2*d_g
    loss = (np.log(denom).sum() - 2.0 * pos_sum) / TWO_N
    return np.array(loss, dtype=np.float32)
